# revision 64
# baseline (speedup 1.0000x reference)
"""Causal self-attention (q/k-swapped variant) Bass kernel for Trainium2.

Problem: B=2, T=2048, C=768, H=12, hs=64.
    k = x@Wk+bk ; q = x@Wq+bq ; v = x@Wv+bv          (per-head split)
    att[b,h,i,j] = (k[b,i,h,:] . q[b,j,h,:]) / 8     (note: k rows, q cols)
    att = softmax(causal-mask(att), axis=j)
    y = (att @ v) @ Wo + bo

Sharding: 8 cores = 2 batches x 4 head-groups (3 heads each).
Each core computes its 3 heads fully (QKV proj -> attention -> partial
output projection); host sums the 4 fp16 partial outputs per batch, adds bo.

Design notes (per core):
- Transposed score space: score tiles have j (the softmax axis) on PSUM
  partitions, i on the free dim, so PV needs no transposes and the softmax
  denominator falls out of the PV matmul via an appended ones-column on V.
- The i range is processed in four 512-col windows, each (window, head)
  unit running j-blocks in PAIRS: jb0's 64-row score matmul runs on PE
  array rows 0-63, jb1's concurrently on rows 64-127 (row-tiled via
  partition placement; partition-swapped Q/K copies in QKs_sb feed the
  upper half). One wide exp per pair ([128, ~1024] PSUM read) halves the
  ACT instruction overhead; pair S tiles are double-buffered (2+2 banks).
- Output projection contracts h0+h1 in a single K=128 matmul (their
  normalized outputs live stacked on AT01's partitions; h1 is shifted up
  via a small DMA) plus one K=64 matmul for h2; blocks run in pairs so
  same-bank accumulation drains hide under the partner block's matmul.
- QKV projection / output projection run as "filler" work inside the
  ACT-paced attention stream (deque with emission-order hazard deadlines);
  a single N=512 dummy matmul per empty filler slot keeps the PE activity
  monitor (HAM clock gate) at full clock.
- The final column chunk is drained in stages: its K=128 matmuls + the
  rank-1 1/Z broadcast run while the last norm chain resolves.
- I/O rides three parallel DMA queues (sync + scalar HWDGE, gpsimd SWDGE),
  ordered so the first window's inputs land first; y is stored as fp16.
"""

import os
import sys

sys.path.insert(0, "/opt/trn_rl_repo")

import numpy as np

T = 2048
C = 768
HS = 64
HPC = 3          # heads per core
NCH = C // 128   # 6 contraction chunks
TB = T // 128    # 16 row blocks
JB = T // 128    # 16 j blocks
NCORES = 8
MM_DTYPE = os.environ.get("KERNEL_MM_DTYPE", "fp16")  # fp16 | bf16 | fp32

_cache = {}


def _segments(lo, hi):
    """Split [lo, hi) at 512 boundaries (PSUM bank / fp32 matmul N limit)."""
    out = []
    s = lo
    while s < hi:
        e = min((s // 512 + 1) * 512, hi)
        out.append((s, e))
        s = e
    return out


def _emit(ctx, tc):
    import concourse.bass as bass
    import concourse.tile as tile  # noqa: F401
    from concourse import mybir
    from concourse.bass import ts
    from concourse.masks import make_upper_triangular

    f32 = mybir.dt.float32
    mmd = {"fp16": mybir.dt.float16, "bf16": mybir.dt.bfloat16,
           "fp32": f32}[MM_DTYPE]  # matmul-input dtype
    nc = tc.nc

    xT = nc.dram_tensor("xT", (C, T), mmd, kind="ExternalInput").ap()
    wqk = nc.dram_tensor("wqk", (128, 3 * NCH * 128), mmd, kind="ExternalInput").ap()
    wv = nc.dram_tensor("wv", (128, NCH * 192), mmd, kind="ExternalInput").ap()
    wo = nc.dram_tensor("wo", (192, C), mmd, kind="ExternalInput").ap()
    bqk = nc.dram_tensor("bqk", (128, 3), f32, kind="ExternalInput").ap()
    bv = nc.dram_tensor("bv", (1, 192), f32, kind="ExternalInput").ap()
    y = nc.dram_tensor("y", (C, T), mmd, kind="ExternalOutput").ap()  # transposed

    consts = ctx.enter_context(tc.tile_pool(name="consts", bufs=1))

    # ---- load inputs (wqk + xT first: they gate the first projections) ----
    # Two HWDGE queues (sync + scalar) in parallel; pieces ordered to match
    # consumption: wqk g1/g0 + xT it01 chunks first (gate qk_group(1,0)...),
    # wv early on scalar (gates v_group/PV).
    wqk_r = wqk.rearrange("p (g k m) -> p g k m", g=3, k=NCH)
    wqk_sb = consts.tile([128, 3, NCH, 128], mmd)
    xT_sb = consts.tile([128, NCH, T], mmd)
    wv_sb = consts.tile([128, NCH, 192], mmd)
    wo01_sb = consts.tile([128, C], mmd)      # Wo rows 0:128 of head-slice
    wo2_sb = consts.tile([64, C], mmd)        # Wo rows 128:192
    bqk_sb = consts.tile([128, 3], f32)       # per-partition bias per QK group
    bvb_sb = consts.tile([128, 192], f32)     # bv broadcast across partitions

    nc.sync.dma_start(wqk_sb[:, 1], wqk_r[:, 1])
    nc.scalar.dma_start(wv_sb[:], wv.rearrange("p (k m) -> p k m", k=NCH))
    for k in range(2):
        nc.sync.dma_start(xT_sb[:, k, 0:1024], xT[k * 128:(k + 1) * 128, 0:1024])
        nc.scalar.dma_start(xT_sb[:, k + 3, 0:1024],
                            xT[(k + 3) * 128:(k + 4) * 128, 0:1024])
    # chunks 2 and 5 of the front half ride the gpsimd queue (its it23
    # cargo isn't consumed until w2): all three streams feed the ramp
    nc.gpsimd.dma_start(xT_sb[:, 2, 0:1024], xT[256:384, 0:1024])
    nc.gpsimd.dma_start(xT_sb[:, 5, 0:1024], xT[640:768, 0:1024])
    nc.sync.dma_start(wqk_sb[:, 0], wqk_r[:, 0])
    nc.scalar.dma_start(bqk_sb[:], bqk)
    nc.scalar.dma_start(bvb_sb[:], bv.to_broadcast((128, 192)))
    # back-half xT via the gpsimd SWDGE queue: a third parallel DMA stream
    for k in range(NCH):
        nc.gpsimd.dma_start(xT_sb[:, k, 1024:2048],
                            xT[k * 128:(k + 1) * 128, 1024:2048])
    nc.sync.dma_start(wqk_sb[:, 2], wqk_r[:, 2])
    nc.scalar.dma_start(wo01_sb[:], wo[0:128, 0:C])
    nc.scalar.dma_start(wo2_sb[:], wo[128:192, 0:C])

    scratch = consts.tile([128, 512], mmd)
    nc.vector.memset(scratch[:], 0.0)
    ones64 = consts.tile([1, 64], f32)
    nc.vector.memset(ones64[:], 1.0)
    trimask = consts.tile([128, 128], mmd)
    make_upper_triangular(nc, trimask[:], val=1.0, diag=True)

    V_aug = consts.tile([128, TB, HPC * 65], mmd)
    for h in range(HPC):
        nc.vector.memset(V_aug[:, :, h * 65 + 64:h * 65 + 65], 1.0)

    QK_sb = consts.tile([128, 3, T], mmd)     # g0=Q(h0,h1) g1=K(h0,h1) g2=[Q(h2)|K(h2)]
    QKs_sb = consts.tile([128, 3, T], mmd)    # partition-swapped copies of QK_sb
    AT01 = consts.tile([128, T], mmd)         # normalized attn out: h0 lo / h1 hi
    AT2 = consts.tile([64, T], mmd)           # h2
    AT1s = consts.tile([64, T], mmd)          # h1 staging (pre partition-shift)

    # ---- single fused pipeline ----
    # PSUM: psP (proj/outproj, 2 banks) + psS (score pairs, 2x2 banks) +
    # psO (Onum, 2x1 bank) = 8
    psP = ctx.enter_context(tc.tile_pool(name="psP", bufs=2, space="PSUM"))
    psS = ctx.enter_context(tc.tile_pool(name="psS", bufs=2, space="PSUM"))
    psO = ctx.enter_context(tc.tile_pool(name="psO", bufs=2, space="PSUM"))
    sbE = ctx.enter_context(tc.tile_pool(name="E", bufs=5))
    sbATn = ctx.enter_context(tc.tile_pool(name="ATn", bufs=2))
    sbRZ = ctx.enter_context(tc.tile_pool(name="RZ", bufs=2))
    sbY = ctx.enter_context(tc.tile_pool(name="Y", bufs=4))

    # PE warm-up (keeps HAM at full clock while inputs stream in) + exp
    # table pre-load
    for _ in range(20):
        warm = psP.tile([128, 512], f32, tag="p")
        nc.tensor.matmul(warm[:], lhsT=scratch[:, 0:128], rhs=scratch[:],
                         start=True, stop=True, skip_group_check=True)
    edum = sbE.tile([128, 1024], mmd, tag="E")
    nc.scalar.activation(edum[:, 0:512], scratch[:],
                         mybir.ActivationFunctionType.Exp, scale=0.125)

    def qk_group(g, it):
        ps = psP.tile([128, 512], f32, tag="p")
        for k in range(NCH):
            nc.tensor.matmul(ps[:], lhsT=wqk_sb[:, g, k, :],
                             rhs=xT_sb[:, k, ts(it, 512)],
                             start=(k == 0), stop=(k == NCH - 1))
        nc.vector.tensor_add(QK_sb[:, g, ts(it, 512)], ps[:],
                             bqk_sb[:, g:g + 1].to_broadcast((128, 512)))
        # partition-swapped copy: both array halves get every head's Q/K so
        # consecutive jb score matmuls can row-tile onto disjoint halves
        nc.sync.dma_start(QKs_sb[64:128, g, ts(it, 512)],
                          QK_sb[0:64, g, ts(it, 512)])
        nc.sync.dma_start(QKs_sb[0:64, g, ts(it, 512)],
                          QK_sb[64:128, g, ts(it, 512)])

    def v_group(tb):
        ps = psP.tile([128, 512], f32, tag="p")
        for k in range(NCH):
            nc.tensor.matmul(ps[:, 0:192], lhsT=xT_sb[:, k, ts(tb, 128)],
                             rhs=wv_sb[:, k, :],
                             start=(k == 0), stop=(k == NCH - 1))
        for h in range(HPC):
            nc.any.tensor_add(V_aug[:, tb, h * 65:h * 65 + 64],
                              ps[:, h * 64:(h + 1) * 64],
                              bvb_sb[:, h * 64:(h + 1) * 64])

    def oproj_pair(cb0, tt):
        # two column-blocks per call; MM1(K=128: h0+h1 stacked) pairs hide
        # MM2's same-bank accumulation drain behind the other block's MM1
        pss = []
        for cb in (cb0, cb0 + 1):
            ps = psP.tile([128, 512], f32, tag="p")
            nc.tensor.matmul(ps[:], lhsT=wo01_sb[:, ts(cb, 128)],
                             rhs=AT01[:, ts(tt, 512)], start=True, stop=False)
            pss.append(ps)
        for cb, ps in zip((cb0, cb0 + 1), pss):
            nc.tensor.matmul(ps[:], lhsT=wo2_sb[:, ts(cb, 128)],
                             rhs=AT2[:, ts(tt, 512)], start=False, stop=True)
        for cb, ps in zip((cb0, cb0 + 1), pss):
            ysb = sbY.tile([128, 512], mmd)
            nc.vector.tensor_copy(ysb[:], ps[:])
            # second-half stores ride gpsimd (idle by then) so sync stays
            # clear for the tail norm chains' latency-critical z DMAs
            dmae = nc.gpsimd if tt >= 2 else nc.sync
            dmae.dma_start(
                y[cb * 128:(cb + 1) * 128, tt * 512:(tt + 1) * 512], ysb[:])

    # pre-phase: K^T cols 0:1023 of h0/h1, Q^T cols for jb 0-3, 3 V blocks;
    # everything else weaves into the chunk stream as PE filler. PV trails
    # ST by 3 chunks and reads V(jb): every V filler below must be emitted
    # no later than the chunk whose (possibly epilogue) PV consumes it.
    qk_group(1, 0)
    qk_group(0, 0)
    for tb in range(4):
        v_group(tb)

    from collections import deque
    # filler schedule for the 512-col-window unit order (w0..w3)x(h0,h1,h2).
    # Forward-hazard deadlines (Tile deps are emission-order based):
    #   qk(g,it) before the first unit whose scores read those columns
    #   v(tb) before the first PV that reads it (window w reads tb <= 4w+3)
    pre_fillers = deque(
        [("qk01", lambda: qk_group(0, 1)), ("qk11", lambda: qk_group(1, 1))]
        + [(f"v{tb}", lambda tb=tb: v_group(tb)) for tb in range(4, 8)]
        + [("qk20", lambda: qk_group(2, 0)), ("qk21", lambda: qk_group(2, 1)),
           ("qk02", lambda: qk_group(0, 2)), ("qk12", lambda: qk_group(1, 2))]
        + [(f"v{tb}", lambda tb=tb: v_group(tb)) for tb in range(8, 12)]
        + [("qk22", lambda: qk_group(2, 2)),
           ("qk03", lambda: qk_group(0, 3)), ("qk13", lambda: qk_group(1, 3))]
        + [(f"v{tb}", lambda tb=tb: v_group(tb)) for tb in range(12, TB)]
        + [("qk23", lambda: qk_group(2, 3))]
    )
    op_fillers = deque()
    emitted = set()
    finals = {}

    def pop_filler():
        if pre_fillers:
            nm, fn = pre_fillers.popleft()
            emitted.add(nm)
            fn()
            return True
        if op_fillers:
            op_fillers.popleft()()
            return True
        return False

    def ensure(name):
        while name not in emitted and pre_fillers:
            nm, fn = pre_fillers.popleft()
            emitted.add(nm)
            fn()

    # per-head (Q^T lo, Q^T hi, K^T lo, K^T hi): lo tiles feed even-jb score
    # matmuls on array rows 0-63, hi tiles feed odd-jb on rows 64-127
    heads = [
        (QK_sb[0:64, 0, :], QKs_sb[64:128, 0, :],
         QK_sb[0:64, 1, :], QKs_sb[64:128, 1, :]),
        (QKs_sb[0:64, 0, :], QK_sb[64:128, 0, :],
         QKs_sb[0:64, 1, :], QK_sb[64:128, 1, :]),
        (QK_sb[0:64, 2, :], QKs_sb[64:128, 2, :],
         QKs_sb[0:64, 2, :], QK_sb[64:128, 2, :]),
    ]

    WN = 512  # i-window per (window, head) unit
    for w in range(T // WN):
        c0 = WN * w
        njb = (c0 + WN) // 128
        last_unit = (w == T // WN - 1)
        for h in range(HPC):
            if h == 0 and w >= 1:
                ensure(f"qk0{w}")
                ensure(f"qk1{w}")
            if h == 2:
                ensure(f"qk2{w}")
            QTlo, QThi, KTlo, KThi = heads[h]
            Onum = psO.tile([65, WN], f32, tag="Onum")
            ATn = sbATn.tile([65, WN], f32)

            def norm(h=h, c0=c0, Onum=Onum, ATn=ATn, last_unit=last_unit):
                # normalize Onum: Z (row 64) -> DMA-reshape -> reciprocal ->
                # fold back -> broadcast across partitions -> divide
                final = last_unit and h == 2
                dmae = nc.scalar if final else nc.sync
                cols = slice(c0, c0 + WN)
                tt = c0 // WN
                nc.vector.tensor_copy(ATn[:], Onum[:])
                z16 = sbRZ.tile([128, 4], f32, tag="z16")
                dmae.dma_start(z16[:], ATn[64:65, :])
                r16 = sbRZ.tile([128, 4], f32, tag="r16")
                nc.vector.reciprocal(r16[:], z16[:])
                rz1 = sbRZ.tile([1, 512], f32, tag="rz1")
                dmae.dma_start(rz1[:], r16[:])
                if final:
                    # broadcast + divide are deferred into the staged drain
                    # (a rank-1 PE matmul) so its h0/h1 matmuls can run
                    # while this chain resolves
                    finals["rz1"] = rz1
                    finals["ATn"] = ATn
                    return
                rzb = sbRZ.tile([64, 512], f32, tag="rzb")
                nc.gpsimd.partition_broadcast(rzb[:], rz1[:], channels=64)
                if h == 0:
                    nc.vector.tensor_mul(AT01[0:64, cols], ATn[0:64, :],
                                         rzb[:])
                elif h == 1:
                    nc.vector.tensor_mul(AT1s[:, cols], ATn[0:64, :], rzb[:])
                    nc.sync.dma_start(AT01[64:128, cols], AT1s[:, cols])
                else:
                    nc.vector.tensor_mul(AT2[:, cols], ATn[0:64, :], rzb[:])
                    # all three heads now normalized for column chunk tt:
                    # its output projection becomes filler work (the final
                    # chunk is instead handled by the staged drain below)
                    for cb in (0, 2, 4):
                        op_fillers.append(
                            lambda cb=cb, tt=tt: oproj_pair(cb, tt))

            def emit_pv(jb, E, eoff, lo, h=h, c0=c0, njb=njb, Onum=Onum):
                # eoff: column offset of this jb's window inside the E tile
                nc.tensor.matmul(Onum[:, lo - c0:],
                                 lhsT=V_aug[:, jb, h * 65:(h + 1) * 65],
                                 rhs=E[:, eoff + lo - c0:eoff + WN],
                                 start=(jb == 0), stop=(jb == njb - 1),
                                 skip_group_check=True)

            trail = 1 if (last_unit and h == 2) else 2
            pending = []
            for p in range(njb // 2):
                jb0, jb1 = 2 * p, 2 * p + 1
                lo0 = max(c0, 128 * jb0)
                lo1 = max(c0, 128 * jb1)
                # jb0 scores on array rows 0-63, jb1 on rows 64-127: the
                # row-tiled matmuls execute concurrently on the PE array
                S = psS.tile([128, 2 * WN], f32, tag="S")
                nc.tensor.matmul(S[:, lo0 - c0:WN],
                                 lhsT=QTlo[:, ts(jb0, 128)],
                                 rhs=KTlo[:, lo0:c0 + WN],
                                 start=True, stop=True)
                nc.tensor.matmul(S[:, WN + lo1 - c0:2 * WN],
                                 lhsT=QThi[:, ts(jb1, 128)],
                                 rhs=KThi[:, lo1:c0 + WN],
                                 start=True, stop=True)
                # one wide exp covers both windows (junk in the gap is never
                # read); halves the ACT instruction overhead
                E = sbE.tile([128, 2 * WN], mmd, tag="E")
                nc.scalar.activation(E[:, lo0 - c0:], S[:, lo0 - c0:],
                                     mybir.ActivationFunctionType.Exp,
                                     scale=0.125)
                if lo0 == 128 * jb0:  # diagonal blocks: mask strict upper part
                    r = 128 * jb0 - c0
                    nc.vector.tensor_mul(E[:, r:r + 128], E[:, r:r + 128],
                                         trimask[:])
                if lo1 == 128 * jb1:
                    r = WN + 128 * jb1 - c0
                    nc.vector.tensor_mul(E[:, r:r + 128], E[:, r:r + 128],
                                         trimask[:])
                if not pop_filler():
                    # dummy full-array matmul: holds the PE activity monitor
                    # at full clock through ACT-paced attention stretches
                    # (sized so S + PV + dummy ~= the 1.15us exp pace)
                    warm = psP.tile([128, 512], f32, tag="p")
                    nc.tensor.matmul(warm[:], lhsT=scratch[:, 0:128],
                                     rhs=scratch[:], start=True, stop=True,
                                     skip_group_check=True)
                pending.append((p, E))
                if len(pending) > trail:
                    pp, EE = pending.pop(0)
                    emit_pv(2 * pp, EE, 0, max(c0, 256 * pp))
                    emit_pv(2 * pp + 1, EE, WN, max(c0, 256 * pp + 128))
            for pp, EE in pending:
                emit_pv(2 * pp, EE, 0, max(c0, 256 * pp))
                pop_filler()
                emit_pv(2 * pp + 1, EE, WN, max(c0, 256 * pp + 128))
            norm()

    while pre_fillers or op_fillers:
        pop_filler()
        # clock-keeper between drained fillers: the last op pairs gate on
        # norm chains and the PE otherwise idles into a HAM re-throttle
        warm = psP.tile([128, 512], f32, tag="p")
        nc.tensor.matmul(warm[:], lhsT=scratch[:, 0:128], rhs=scratch[:],
                         start=True, stop=True, skip_group_check=True)

    # staged drain of the final column chunk (tt=3): the K=128 h0/h1 matmuls
    # don't depend on the last norm chain -- run them (plus a few clock-keeper
    # dummies) while it resolves, leaving only the K=64 h2 accumulation
    tt = T // 512 - 1
    big0 = psS.tile([128, 2 * WN], f32, name="drain0", tag="S")
    big1 = psS.tile([128, 2 * WN], f32, name="drain1", tag="S")
    pss = []
    for cb in range(NCH):
        if cb < 4:
            ps = (big0 if cb < 2 else big1)[:, 512 * (cb % 2):512 * (cb % 2 + 1)]
        else:
            ps = psP.tile([128, 512], f32, tag="p")
        nc.tensor.matmul(ps, lhsT=wo01_sb[:, ts(cb, 128)],
                         rhs=AT01[:, ts(tt, 512)], start=True, stop=False)
        pss.append(ps)
    dum = psO.tile([65, WN], f32, tag="Onum")
    for _ in range(5):
        nc.tensor.matmul(dum[:, 0:512], lhsT=scratch[:, 0:65],
                         rhs=scratch[:], start=True, stop=True,
                         skip_group_check=True)
    # deferred tail of the final norm: replicate 1/Z across 64 partitions
    # with a rank-1 matmul (into the spare Onum psum slot), then divide
    rzb_ps = psO.tile([65, WN], f32, tag="Onum")
    nc.tensor.matmul(rzb_ps[0:64, :], lhsT=ones64[:], rhs=finals["rz1"][:],
                     start=True, stop=True, skip_group_check=True)
    nc.vector.tensor_mul(AT2[:, tt * 512:(tt + 1) * 512],
                         finals["ATn"][0:64, :], rzb_ps[0:64, :])
    for cb in range(NCH):
        nc.tensor.matmul(pss[cb], lhsT=wo2_sb[:, ts(cb, 128)],
                         rhs=AT2[:, ts(tt, 512)], start=False, stop=True)
        ysb = sbY.tile([128, 512], mmd)
        nc.vector.tensor_copy(ysb[:], pss[cb])
        dmae = nc.scalar if cb % 2 else nc.sync
        dmae.dma_start(y[cb * 128:(cb + 1) * 128, tt * 512:(tt + 1) * 512],
                       ysb[:])


def _build():
    if "nc" in _cache:
        return _cache["nc"]
    from contextlib import ExitStack

    import concourse.tile as tile
    from concourse import bacc

    nc = bacc.Bacc("TRN2", target_bir_lowering=False, debug=False,
                   num_devices=NCORES)
    with tile.TileContext(nc) as tc:
        with ExitStack() as ctx:
            _emit(ctx, tc)
    nc.compile()
    _cache["nc"] = nc
    return nc


def _install_trace_hooks():
    """Make trace=True work in this container: shim the missing
    antenv.axon_hooks NTFF-profile hook (ctypes into libaxon_pjrt.so) and
    skip the S3 artifact upload."""
    import contextlib
    import ctypes
    import types

    import concourse.bass_utils as bu

    bu.upload_artifacts = lambda tmpdir: tmpdir
    try:
        from antenv.axon_hooks import get_axon_ntff_profile_hook  # noqa: F401
        return
    except ImportError:
        pass

    so_path = "/opt/axon/libaxon_pjrt.so"
    if not os.path.exists(so_path):
        return
    lib = ctypes.CDLL(so_path)
    if not hasattr(lib, "axon_start_nrt_profile"):
        return
    lib.axon_start_nrt_profile.argtypes = [
        ctypes.POINTER(ctypes.c_int64), ctypes.c_size_t,
    ]
    lib.axon_start_nrt_profile.restype = ctypes.c_int64
    lib.axon_stop_nrt_profile.argtypes = [ctypes.c_char_p]
    lib.axon_stop_nrt_profile.restype = ctypes.c_int64

    @contextlib.contextmanager
    def _hook(output_dir, device_ids):
        import jax
        jax.devices()
        if device_ids:
            ids = (ctypes.c_int64 * len(device_ids))(*device_ids)
            rc = lib.axon_start_nrt_profile(ids, len(device_ids))
        else:
            rc = lib.axon_start_nrt_profile(None, 0)
        if rc != 0:
            raise RuntimeError(f"axon_start_nrt_profile rc={rc}")
        try:
            yield
        finally:
            n = lib.axon_stop_nrt_profile(str(output_dir).encode())
            print(f"profile: {n} file(s) written to {output_dir}",
                  file=sys.stderr)

    state = {"h": _hook}
    mod = types.ModuleType("antenv.axon_hooks")
    mod.get_axon_ntff_profile_hook = lambda: state["h"]
    mod.set_axon_ntff_profile_hook = lambda h: state.__setitem__("h", h)
    import antenv
    antenv.axon_hooks = mod
    sys.modules["antenv.axon_hooks"] = mod


def kernel(**inputs):
    x = np.ascontiguousarray(np.asarray(inputs["x"], dtype=np.float32))
    Wq = np.asarray(inputs["Wq"], dtype=np.float32)
    Wk = np.asarray(inputs["Wk"], dtype=np.float32)
    Wv = np.asarray(inputs["Wv"], dtype=np.float32)
    Wo = np.asarray(inputs["Wo"], dtype=np.float32)
    bq = np.asarray(inputs["bq"], dtype=np.float32)
    bk = np.asarray(inputs["bk"], dtype=np.float32)
    bv = np.asarray(inputs["bv"], dtype=np.float32)
    bo = np.asarray(inputs["bo"], dtype=np.float32)

    from concourse import bass_utils

    nc = _build()

    if MM_DTYPE == "bf16":
        import ml_dtypes
        mmd_np = ml_dtypes.bfloat16
    elif MM_DTYPE == "fp16":
        mmd_np = np.float16
    else:
        mmd_np = np.float32

    B = x.shape[0]
    xTs = [np.ascontiguousarray(x[b].T.astype(mmd_np)) for b in range(B)]
    in_maps = []
    for core in range(NCORES):
        b, hg = core // 4, core % 4
        sl = slice(hg * 192, (hg + 1) * 192)
        wq_s, wk_s = Wq[:, sl], Wk[:, sl]
        g0 = wq_s[:, 0:128]
        g1 = wk_s[:, 0:128]
        g2 = np.concatenate([wq_s[:, 128:192], wk_s[:, 128:192]], axis=1)
        wqk_h = (np.stack([g0, g1, g2], 0)
                 .reshape(3, NCH, 128, 128).transpose(2, 0, 1, 3)
                 .reshape(128, 3 * NCH * 128))
        wv_h = (Wv[:, sl].reshape(NCH, 128, 192).transpose(1, 0, 2)
                .reshape(128, NCH * 192))
        wo_h = Wo[sl, :]  # (192, C): rows 0:128 = h0|h1 stacked, 128:192 = h2
        bqk_h = np.stack(
            [bq[sl][0:128], bk[sl][0:128],
             np.concatenate([bq[sl][128:192], bk[sl][128:192]])], axis=1
        )  # [128, 3]
        bv_h = bv[sl].reshape(1, 192)
        in_maps.append({
            "xT": xTs[b],
            "wqk": np.ascontiguousarray(wqk_h.astype(mmd_np)),
            "wv": np.ascontiguousarray(wv_h.astype(mmd_np)),
            "wo": np.ascontiguousarray(wo_h.astype(mmd_np)),
            "bqk": np.ascontiguousarray(bqk_h),
            "bv": np.ascontiguousarray(bv_h),
        })

    trace = bool(os.environ.get("KERNEL_TRACE"))
    if trace:
        _install_trace_hooks()
    res = bass_utils.run_bass_kernel_spmd(
        nc, in_maps, core_ids=list(range(NCORES)), trace=trace
    )
    _cache["last_results"] = res

    out = np.empty((B, T, C), dtype=np.float32)
    for b in range(B):
        acc = res.results[b * 4]["y"].astype(np.float32)
        for hg in range(1, 4):
            acc += res.results[b * 4 + hg]["y"].astype(np.float32)
        out[b] = acc.T + bo
    return out



# revision 66
# speedup vs baseline: 1.0197x; 1.0197x over previous
"""Causal self-attention (q/k-swapped variant) Bass kernel for Trainium2.

Problem: B=2, T=2048, C=768, H=12, hs=64.
    k = x@Wk+bk ; q = x@Wq+bq ; v = x@Wv+bv          (per-head split)
    att[b,h,i,j] = (k[b,i,h,:] . q[b,j,h,:]) / 8     (note: k rows, q cols)
    att = softmax(causal-mask(att), axis=j)
    y = (att @ v) @ Wo + bo

Sharding: 8 cores = 2 batches x 4 head-groups (3 heads each).
Each core computes its 3 heads fully (QKV proj -> attention -> partial
output projection); host sums the 4 fp16 partial outputs per batch, adds bo.

Design notes (per core):
- Transposed score space: score tiles have j (the softmax axis) on PSUM
  partitions, i on the free dim, so PV needs no transposes and the softmax
  denominator falls out of the PV matmul via an appended ones-column on V.
- The i range is processed in four 512-col windows, each (window, head)
  unit running j-blocks in PAIRS: jb0's 64-row score matmul runs on PE
  array rows 0-63, jb1's concurrently on rows 64-127 (row-tiled via
  partition placement; partition-swapped Q/K copies in QKs_sb feed the
  upper half). One wide exp per pair ([128, ~1024] PSUM read) halves the
  ACT instruction overhead; pair S tiles are double-buffered (2+2 banks).
- Output projection contracts h0+h1 in a single K=128 matmul (their
  normalized outputs live stacked on AT01's partitions; h1 is shifted up
  via a small DMA) plus one K=64 matmul for h2; blocks run in pairs so
  same-bank accumulation drains hide under the partner block's matmul.
- QKV projection / output projection run as "filler" work inside the
  ACT-paced attention stream (deque with emission-order hazard deadlines);
  a single N=512 dummy matmul per empty filler slot keeps the PE activity
  monitor (HAM clock gate) at full clock.
- The final column chunk is drained in stages: its K=128 matmuls + the
  rank-1 1/Z broadcast run while the last norm chain resolves.
- I/O rides three parallel DMA queues (sync + scalar HWDGE, gpsimd SWDGE),
  ordered so the first window's inputs land first; y is stored as fp16.
"""

import os
import sys

sys.path.insert(0, "/opt/trn_rl_repo")

import numpy as np

T = 2048
C = 768
HS = 64
HPC = 3          # heads per core
NCH = C // 128   # 6 contraction chunks
TB = T // 128    # 16 row blocks
JB = T // 128    # 16 j blocks
NCORES = 8
MM_DTYPE = os.environ.get("KERNEL_MM_DTYPE", "fp16")  # fp16 | bf16 | fp32

_cache = {}


def _segments(lo, hi):
    """Split [lo, hi) at 512 boundaries (PSUM bank / fp32 matmul N limit)."""
    out = []
    s = lo
    while s < hi:
        e = min((s // 512 + 1) * 512, hi)
        out.append((s, e))
        s = e
    return out


def _emit(ctx, tc):
    import concourse.bass as bass
    import concourse.tile as tile  # noqa: F401
    from concourse import mybir
    from concourse.bass import ts
    from concourse.masks import make_upper_triangular

    f32 = mybir.dt.float32
    mmd = {"fp16": mybir.dt.float16, "bf16": mybir.dt.bfloat16,
           "fp32": f32}[MM_DTYPE]  # matmul-input dtype
    nc = tc.nc

    xT = nc.dram_tensor("xT", (C, T), mmd, kind="ExternalInput").ap()
    wqk = nc.dram_tensor("wqk", (128, 3 * NCH * 128), mmd, kind="ExternalInput").ap()
    wv = nc.dram_tensor("wv", (128, NCH * 192), mmd, kind="ExternalInput").ap()
    wo = nc.dram_tensor("wo", (192, C), mmd, kind="ExternalInput").ap()
    bqk = nc.dram_tensor("bqk", (128, 3), f32, kind="ExternalInput").ap()
    bv = nc.dram_tensor("bv", (1, 192), f32, kind="ExternalInput").ap()
    y = nc.dram_tensor("y", (C, T), mmd, kind="ExternalOutput").ap()  # transposed

    consts = ctx.enter_context(tc.tile_pool(name="consts", bufs=1))

    # ---- load inputs (wqk + xT first: they gate the first projections) ----
    # Two HWDGE queues (sync + scalar) in parallel; pieces ordered to match
    # consumption: wqk g1/g0 + xT it01 chunks first (gate qk_group(1,0)...),
    # wv early on scalar (gates v_group/PV).
    wqk_r = wqk.rearrange("p (g k m) -> p g k m", g=3, k=NCH)
    wqk_sb = consts.tile([128, 3, NCH, 128], mmd)
    xT_sb = consts.tile([128, NCH, T], mmd)
    wv_sb = consts.tile([128, NCH, 192], mmd)
    wo01_sb = consts.tile([128, C], mmd)      # Wo rows 0:128 of head-slice
    wo2_sb = consts.tile([64, C], mmd)        # Wo rows 128:192
    bqk_sb = consts.tile([128, 3], f32)       # per-partition bias per QK group
    bvb_sb = consts.tile([128, 192], f32)     # bv broadcast across partitions

    nc.sync.dma_start(wqk_sb[:, 1], wqk_r[:, 1])
    nc.scalar.dma_start(wv_sb[:], wv.rearrange("p (k m) -> p k m", k=NCH))
    for k in range(2):
        nc.sync.dma_start(xT_sb[:, k, 0:1024], xT[k * 128:(k + 1) * 128, 0:1024])
        nc.scalar.dma_start(xT_sb[:, k + 3, 0:1024],
                            xT[(k + 3) * 128:(k + 4) * 128, 0:1024])
    # chunks 2 and 5 of the front half ride the gpsimd queue (its it23
    # cargo isn't consumed until w2): all three streams feed the ramp
    nc.gpsimd.dma_start(xT_sb[:, 2, 0:1024], xT[256:384, 0:1024])
    nc.gpsimd.dma_start(xT_sb[:, 5, 0:1024], xT[640:768, 0:1024])
    nc.sync.dma_start(wqk_sb[:, 0], wqk_r[:, 0])
    nc.scalar.dma_start(bqk_sb[:], bqk)
    nc.scalar.dma_start(bvb_sb[:], bv.to_broadcast((128, 192)))
    # back-half xT via the gpsimd SWDGE queue: a third parallel DMA stream
    for k in range(NCH):
        nc.gpsimd.dma_start(xT_sb[:, k, 1024:2048],
                            xT[k * 128:(k + 1) * 128, 1024:2048])
    nc.sync.dma_start(wqk_sb[:, 2], wqk_r[:, 2])
    nc.scalar.dma_start(wo01_sb[:], wo[0:128, 0:C])
    nc.scalar.dma_start(wo2_sb[:], wo[128:192, 0:C])

    scratch = consts.tile([128, 512], mmd)
    nc.vector.memset(scratch[:], 0.0)
    ones64 = consts.tile([1, 64], f32)
    nc.vector.memset(ones64[:], 1.0)
    trimask = consts.tile([128, 128], mmd)
    make_upper_triangular(nc, trimask[:], val=1.0, diag=True)

    V_aug = consts.tile([128, TB, HPC * 65], mmd)
    for h in range(HPC):
        nc.vector.memset(V_aug[:, :, h * 65 + 64:h * 65 + 65], 1.0)

    QK_sb = consts.tile([128, 3, T], mmd)     # g0=Q(h0,h1) g1=K(h0,h1) g2=[Q(h2)|K(h2)]
    QKs_sb = consts.tile([128, 3, T], mmd)    # partition-swapped copies of QK_sb
    AT01 = consts.tile([128, T], mmd)         # normalized attn out: h0 lo / h1 hi
    AT2 = consts.tile([64, T], mmd)           # h2
    AT1s = consts.tile([64, T], mmd)          # h1 staging (pre partition-shift)

    # ---- single fused pipeline ----
    # PSUM: psP (proj/outproj, 2 banks) + psS (score pairs, 2x2 banks) +
    # psO (Onum, 2x1 bank) = 8
    psP = ctx.enter_context(tc.tile_pool(name="psP", bufs=2, space="PSUM"))
    psS = ctx.enter_context(tc.tile_pool(name="psS", bufs=2, space="PSUM"))
    psO = ctx.enter_context(tc.tile_pool(name="psO", bufs=2, space="PSUM"))
    sbE = ctx.enter_context(tc.tile_pool(name="E", bufs=5))
    sbATn = ctx.enter_context(tc.tile_pool(name="ATn", bufs=2))
    sbRZ = ctx.enter_context(tc.tile_pool(name="RZ", bufs=2))
    sbY = ctx.enter_context(tc.tile_pool(name="Y", bufs=4))

    # PE warm-up (keeps HAM at full clock while inputs stream in) + exp
    # table pre-load
    for _ in range(16):
        warm = psP.tile([128, 512], f32, tag="p")
        nc.tensor.matmul(warm[:], lhsT=scratch[:, 0:128], rhs=scratch[:],
                         start=True, stop=True, skip_group_check=True)
    edum = sbE.tile([128, 1024], mmd, tag="E")
    nc.scalar.activation(edum[:, 0:512], scratch[:],
                         mybir.ActivationFunctionType.Exp, scale=0.125)

    def qk_group(g, it):
        ps = psP.tile([128, 512], f32, tag="p")
        for k in range(NCH):
            nc.tensor.matmul(ps[:], lhsT=wqk_sb[:, g, k, :],
                             rhs=xT_sb[:, k, ts(it, 512)],
                             start=(k == 0), stop=(k == NCH - 1))
        nc.vector.tensor_add(QK_sb[:, g, ts(it, 512)], ps[:],
                             bqk_sb[:, g:g + 1].to_broadcast((128, 512)))
        # partition-swapped copy: both array halves get every head's Q/K so
        # consecutive jb score matmuls can row-tile onto disjoint halves
        nc.sync.dma_start(QKs_sb[64:128, g, ts(it, 512)],
                          QK_sb[0:64, g, ts(it, 512)])
        nc.sync.dma_start(QKs_sb[0:64, g, ts(it, 512)],
                          QK_sb[64:128, g, ts(it, 512)])

    def v_group(tb):
        ps = psP.tile([128, 512], f32, tag="p")
        for k in range(NCH):
            nc.tensor.matmul(ps[:, 0:192], lhsT=xT_sb[:, k, ts(tb, 128)],
                             rhs=wv_sb[:, k, :],
                             start=(k == 0), stop=(k == NCH - 1))
        for h in range(HPC):
            nc.any.tensor_add(V_aug[:, tb, h * 65:h * 65 + 64],
                              ps[:, h * 64:(h + 1) * 64],
                              bvb_sb[:, h * 64:(h + 1) * 64])

    def oproj_pair(cb0, tt):
        # two column-blocks per call; MM1(K=128: h0+h1 stacked) pairs hide
        # MM2's same-bank accumulation drain behind the other block's MM1
        pss = []
        for cb in (cb0, cb0 + 1):
            ps = psP.tile([128, 512], f32, tag="p")
            nc.tensor.matmul(ps[:], lhsT=wo01_sb[:, ts(cb, 128)],
                             rhs=AT01[:, ts(tt, 512)], start=True, stop=False)
            pss.append(ps)
        for cb, ps in zip((cb0, cb0 + 1), pss):
            nc.tensor.matmul(ps[:], lhsT=wo2_sb[:, ts(cb, 128)],
                             rhs=AT2[:, ts(tt, 512)], start=False, stop=True)
        for cb, ps in zip((cb0, cb0 + 1), pss):
            ysb = sbY.tile([128, 512], mmd)
            nc.vector.tensor_copy(ysb[:], ps[:])
            nc.sync.dma_start(
                y[cb * 128:(cb + 1) * 128, tt * 512:(tt + 1) * 512], ysb[:])

    # pre-phase: K^T cols 0:1023 of h0/h1, Q^T cols for jb 0-3, 3 V blocks;
    # everything else weaves into the chunk stream as PE filler. PV trails
    # ST by 3 chunks and reads V(jb): every V filler below must be emitted
    # no later than the chunk whose (possibly epilogue) PV consumes it.
    qk_group(1, 0)
    qk_group(0, 0)

    from collections import deque
    # filler schedule for the 512-col-window unit order (w0..w3)x(h0,h1,h2).
    # Forward-hazard deadlines (Tile deps are emission-order based):
    #   qk(g,it) before the first unit whose scores read those columns
    #   v(tb) before the first PV that reads it (window w reads tb <= 4w+3;
    #   v0..v3 land exactly at (w0,h0)'s four pops, the last two of which
    #   sit between its epilogue PV pairs)
    pre_fillers = deque(
        [(f"v{tb}", lambda tb=tb: v_group(tb)) for tb in range(0, 4)]
        + [("qk01", lambda: qk_group(0, 1)), ("qk11", lambda: qk_group(1, 1))]
        + [(f"v{tb}", lambda tb=tb: v_group(tb)) for tb in range(4, 8)]
        + [("qk20", lambda: qk_group(2, 0)), ("qk21", lambda: qk_group(2, 1)),
           ("qk02", lambda: qk_group(0, 2)), ("qk12", lambda: qk_group(1, 2))]
        + [(f"v{tb}", lambda tb=tb: v_group(tb)) for tb in range(8, 12)]
        + [("qk22", lambda: qk_group(2, 2)),
           ("qk03", lambda: qk_group(0, 3)), ("qk13", lambda: qk_group(1, 3))]
        + [(f"v{tb}", lambda tb=tb: v_group(tb)) for tb in range(12, TB)]
        + [("qk23", lambda: qk_group(2, 3))]
    )
    op_fillers = deque()
    emitted = set()
    finals = {}

    def pop_filler():
        if pre_fillers:
            nm, fn = pre_fillers.popleft()
            emitted.add(nm)
            fn()
            return True
        if op_fillers:
            op_fillers.popleft()()
            return True
        return False

    def ensure(name):
        while name not in emitted and pre_fillers:
            nm, fn = pre_fillers.popleft()
            emitted.add(nm)
            fn()

    # per-head (Q^T lo, Q^T hi, K^T lo, K^T hi): lo tiles feed even-jb score
    # matmuls on array rows 0-63, hi tiles feed odd-jb on rows 64-127
    heads = [
        (QK_sb[0:64, 0, :], QKs_sb[64:128, 0, :],
         QK_sb[0:64, 1, :], QKs_sb[64:128, 1, :]),
        (QKs_sb[0:64, 0, :], QK_sb[64:128, 0, :],
         QKs_sb[0:64, 1, :], QK_sb[64:128, 1, :]),
        (QK_sb[0:64, 2, :], QKs_sb[64:128, 2, :],
         QKs_sb[0:64, 2, :], QK_sb[64:128, 2, :]),
    ]

    WN = 512  # i-window per (window, head) unit
    for w in range(T // WN):
        c0 = WN * w
        njb = (c0 + WN) // 128
        last_unit = (w == T // WN - 1)
        for h in range(HPC):
            if h == 0 and w >= 1:
                ensure(f"qk0{w}")
                ensure(f"qk1{w}")
            if h == 2:
                ensure(f"qk2{w}")
            QTlo, QThi, KTlo, KThi = heads[h]
            Onum = psO.tile([65, WN], f32, tag="Onum")
            ATn = sbATn.tile([65, WN], f32)

            def norm(h=h, c0=c0, Onum=Onum, ATn=ATn, last_unit=last_unit):
                # normalize Onum: Z (row 64) -> DMA-reshape -> reciprocal ->
                # fold back -> broadcast across partitions -> divide
                final = last_unit and h == 2
                dmae = nc.scalar if final else nc.sync
                cols = slice(c0, c0 + WN)
                tt = c0 // WN
                nc.vector.tensor_copy(ATn[:], Onum[:])
                z16 = sbRZ.tile([128, 4], f32, tag="z16")
                dmae.dma_start(z16[:], ATn[64:65, :])
                r16 = sbRZ.tile([128, 4], f32, tag="r16")
                nc.vector.reciprocal(r16[:], z16[:])
                rz1 = sbRZ.tile([1, 512], f32, tag="rz1")
                dmae.dma_start(rz1[:], r16[:])
                if final:
                    # broadcast + divide are deferred into the staged drain
                    # (a rank-1 PE matmul) so its h0/h1 matmuls can run
                    # while this chain resolves
                    finals["rz1"] = rz1
                    finals["ATn"] = ATn
                    return
                rzb = sbRZ.tile([64, 512], f32, tag="rzb")
                nc.gpsimd.partition_broadcast(rzb[:], rz1[:], channels=64)
                if h == 0:
                    nc.vector.tensor_mul(AT01[0:64, cols], ATn[0:64, :],
                                         rzb[:])
                elif h == 1:
                    nc.vector.tensor_mul(AT1s[:, cols], ATn[0:64, :], rzb[:])
                    nc.sync.dma_start(AT01[64:128, cols], AT1s[:, cols])
                else:
                    nc.vector.tensor_mul(AT2[:, cols], ATn[0:64, :], rzb[:])
                    # all three heads now normalized for column chunk tt:
                    # its output projection becomes filler work (the final
                    # chunk is instead handled by the staged drain below)
                    for cb in (0, 2, 4):
                        op_fillers.append(
                            lambda cb=cb, tt=tt: oproj_pair(cb, tt))

            def emit_pv(jb, E, eoff, lo, h=h, c0=c0, njb=njb, Onum=Onum):
                # eoff: column offset of this jb's window inside the E tile
                nc.tensor.matmul(Onum[:, lo - c0:],
                                 lhsT=V_aug[:, jb, h * 65:(h + 1) * 65],
                                 rhs=E[:, eoff + lo - c0:eoff + WN],
                                 start=(jb == 0), stop=(jb == njb - 1),
                                 skip_group_check=True)

            trail = 1 if (last_unit and h == 2) else 2
            pending = []
            for p in range(njb // 2):
                jb0, jb1 = 2 * p, 2 * p + 1
                lo0 = max(c0, 128 * jb0)
                lo1 = max(c0, 128 * jb1)
                # jb0 scores on array rows 0-63, jb1 on rows 64-127: the
                # row-tiled matmuls execute concurrently on the PE array
                S = psS.tile([128, 2 * WN], f32, tag="S")
                nc.tensor.matmul(S[:, lo0 - c0:WN],
                                 lhsT=QTlo[:, ts(jb0, 128)],
                                 rhs=KTlo[:, lo0:c0 + WN],
                                 start=True, stop=True)
                nc.tensor.matmul(S[:, WN + lo1 - c0:2 * WN],
                                 lhsT=QThi[:, ts(jb1, 128)],
                                 rhs=KThi[:, lo1:c0 + WN],
                                 start=True, stop=True)
                # one wide exp covers both windows (junk in the gap is never
                # read); halves the ACT instruction overhead
                E = sbE.tile([128, 2 * WN], mmd, tag="E")
                nc.scalar.activation(E[:, lo0 - c0:], S[:, lo0 - c0:],
                                     mybir.ActivationFunctionType.Exp,
                                     scale=0.125)
                if lo0 == 128 * jb0:  # diagonal blocks: mask strict upper part
                    r = 128 * jb0 - c0
                    nc.vector.tensor_mul(E[:, r:r + 128], E[:, r:r + 128],
                                         trimask[:])
                if lo1 == 128 * jb1:
                    r = WN + 128 * jb1 - c0
                    nc.vector.tensor_mul(E[:, r:r + 128], E[:, r:r + 128],
                                         trimask[:])
                if not pop_filler():
                    # dummy full-array matmul: holds the PE activity monitor
                    # at full clock through ACT-paced attention stretches
                    # (sized so S + PV + dummy ~= the 1.15us exp pace)
                    warm = psP.tile([128, 512], f32, tag="p")
                    nc.tensor.matmul(warm[:], lhsT=scratch[:, 0:128],
                                     rhs=scratch[:], start=True, stop=True,
                                     skip_group_check=True)
                pending.append((p, E))
                if len(pending) > trail:
                    pp, EE = pending.pop(0)
                    emit_pv(2 * pp, EE, 0, max(c0, 256 * pp))
                    emit_pv(2 * pp + 1, EE, WN, max(c0, 256 * pp + 128))
            for pp, EE in pending:
                emit_pv(2 * pp, EE, 0, max(c0, 256 * pp))
                pop_filler()
                emit_pv(2 * pp + 1, EE, WN, max(c0, 256 * pp + 128))
            norm()

    while pre_fillers or op_fillers:
        pop_filler()
        # clock-keeper between drained fillers: the last op pairs gate on
        # norm chains and the PE otherwise idles into a HAM re-throttle
        warm = psP.tile([128, 512], f32, tag="p")
        nc.tensor.matmul(warm[:], lhsT=scratch[:, 0:128], rhs=scratch[:],
                         start=True, stop=True, skip_group_check=True)

    # staged drain of the final column chunk (tt=3): the K=128 h0/h1 matmuls
    # don't depend on the last norm chain -- run them (plus a few clock-keeper
    # dummies) while it resolves, leaving only the K=64 h2 accumulation
    tt = T // 512 - 1
    big0 = psS.tile([128, 2 * WN], f32, name="drain0", tag="S")
    big1 = psS.tile([128, 2 * WN], f32, name="drain1", tag="S")
    pss = []
    for cb in range(NCH):
        if cb < 4:
            ps = (big0 if cb < 2 else big1)[:, 512 * (cb % 2):512 * (cb % 2 + 1)]
        else:
            ps = psP.tile([128, 512], f32, tag="p")
        nc.tensor.matmul(ps, lhsT=wo01_sb[:, ts(cb, 128)],
                         rhs=AT01[:, ts(tt, 512)], start=True, stop=False)
        pss.append(ps)
    dum = psO.tile([65, WN], f32, tag="Onum")
    for _ in range(5):
        nc.tensor.matmul(dum[:, 0:512], lhsT=scratch[:, 0:65],
                         rhs=scratch[:], start=True, stop=True,
                         skip_group_check=True)
    # deferred tail of the final norm: replicate 1/Z across 64 partitions
    # with a rank-1 matmul (into the spare Onum psum slot), then divide
    rzb_ps = psO.tile([65, WN], f32, tag="Onum")
    nc.tensor.matmul(rzb_ps[0:64, :], lhsT=ones64[:], rhs=finals["rz1"][:],
                     start=True, stop=True, skip_group_check=True)
    nc.vector.tensor_mul(AT2[:, tt * 512:(tt + 1) * 512],
                         finals["ATn"][0:64, :], rzb_ps[0:64, :])
    for cb in range(NCH):
        nc.tensor.matmul(pss[cb], lhsT=wo2_sb[:, ts(cb, 128)],
                         rhs=AT2[:, ts(tt, 512)], start=False, stop=True)
        ysb = sbY.tile([128, 512], mmd)
        nc.vector.tensor_copy(ysb[:], pss[cb])
        dmae = nc.scalar if cb % 2 else nc.sync
        dmae.dma_start(y[cb * 128:(cb + 1) * 128, tt * 512:(tt + 1) * 512],
                       ysb[:])


def _build():
    if "nc" in _cache:
        return _cache["nc"]
    from contextlib import ExitStack

    import concourse.tile as tile
    from concourse import bacc

    nc = bacc.Bacc("TRN2", target_bir_lowering=False, debug=False,
                   num_devices=NCORES)
    with tile.TileContext(nc) as tc:
        with ExitStack() as ctx:
            _emit(ctx, tc)
    nc.compile()
    _cache["nc"] = nc
    return nc


def _install_trace_hooks():
    """Make trace=True work in this container: shim the missing
    antenv.axon_hooks NTFF-profile hook (ctypes into libaxon_pjrt.so) and
    skip the S3 artifact upload."""
    import contextlib
    import ctypes
    import types

    import concourse.bass_utils as bu

    bu.upload_artifacts = lambda tmpdir: tmpdir
    try:
        from antenv.axon_hooks import get_axon_ntff_profile_hook  # noqa: F401
        return
    except ImportError:
        pass

    so_path = "/opt/axon/libaxon_pjrt.so"
    if not os.path.exists(so_path):
        return
    lib = ctypes.CDLL(so_path)
    if not hasattr(lib, "axon_start_nrt_profile"):
        return
    lib.axon_start_nrt_profile.argtypes = [
        ctypes.POINTER(ctypes.c_int64), ctypes.c_size_t,
    ]
    lib.axon_start_nrt_profile.restype = ctypes.c_int64
    lib.axon_stop_nrt_profile.argtypes = [ctypes.c_char_p]
    lib.axon_stop_nrt_profile.restype = ctypes.c_int64

    @contextlib.contextmanager
    def _hook(output_dir, device_ids):
        import jax
        jax.devices()
        if device_ids:
            ids = (ctypes.c_int64 * len(device_ids))(*device_ids)
            rc = lib.axon_start_nrt_profile(ids, len(device_ids))
        else:
            rc = lib.axon_start_nrt_profile(None, 0)
        if rc != 0:
            raise RuntimeError(f"axon_start_nrt_profile rc={rc}")
        try:
            yield
        finally:
            n = lib.axon_stop_nrt_profile(str(output_dir).encode())
            print(f"profile: {n} file(s) written to {output_dir}",
                  file=sys.stderr)

    state = {"h": _hook}
    mod = types.ModuleType("antenv.axon_hooks")
    mod.get_axon_ntff_profile_hook = lambda: state["h"]
    mod.set_axon_ntff_profile_hook = lambda h: state.__setitem__("h", h)
    import antenv
    antenv.axon_hooks = mod
    sys.modules["antenv.axon_hooks"] = mod


def kernel(**inputs):
    x = np.ascontiguousarray(np.asarray(inputs["x"], dtype=np.float32))
    Wq = np.asarray(inputs["Wq"], dtype=np.float32)
    Wk = np.asarray(inputs["Wk"], dtype=np.float32)
    Wv = np.asarray(inputs["Wv"], dtype=np.float32)
    Wo = np.asarray(inputs["Wo"], dtype=np.float32)
    bq = np.asarray(inputs["bq"], dtype=np.float32)
    bk = np.asarray(inputs["bk"], dtype=np.float32)
    bv = np.asarray(inputs["bv"], dtype=np.float32)
    bo = np.asarray(inputs["bo"], dtype=np.float32)

    from concourse import bass_utils

    nc = _build()

    if MM_DTYPE == "bf16":
        import ml_dtypes
        mmd_np = ml_dtypes.bfloat16
    elif MM_DTYPE == "fp16":
        mmd_np = np.float16
    else:
        mmd_np = np.float32

    B = x.shape[0]
    xTs = [np.ascontiguousarray(x[b].T.astype(mmd_np)) for b in range(B)]
    in_maps = []
    for core in range(NCORES):
        b, hg = core // 4, core % 4
        sl = slice(hg * 192, (hg + 1) * 192)
        wq_s, wk_s = Wq[:, sl], Wk[:, sl]
        g0 = wq_s[:, 0:128]
        g1 = wk_s[:, 0:128]
        g2 = np.concatenate([wq_s[:, 128:192], wk_s[:, 128:192]], axis=1)
        wqk_h = (np.stack([g0, g1, g2], 0)
                 .reshape(3, NCH, 128, 128).transpose(2, 0, 1, 3)
                 .reshape(128, 3 * NCH * 128))
        wv_h = (Wv[:, sl].reshape(NCH, 128, 192).transpose(1, 0, 2)
                .reshape(128, NCH * 192))
        wo_h = Wo[sl, :]  # (192, C): rows 0:128 = h0|h1 stacked, 128:192 = h2
        bqk_h = np.stack(
            [bq[sl][0:128], bk[sl][0:128],
             np.concatenate([bq[sl][128:192], bk[sl][128:192]])], axis=1
        )  # [128, 3]
        bv_h = bv[sl].reshape(1, 192)
        in_maps.append({
            "xT": xTs[b],
            "wqk": np.ascontiguousarray(wqk_h.astype(mmd_np)),
            "wv": np.ascontiguousarray(wv_h.astype(mmd_np)),
            "wo": np.ascontiguousarray(wo_h.astype(mmd_np)),
            "bqk": np.ascontiguousarray(bqk_h),
            "bv": np.ascontiguousarray(bv_h),
        })

    trace = bool(os.environ.get("KERNEL_TRACE"))
    if trace:
        _install_trace_hooks()
    res = bass_utils.run_bass_kernel_spmd(
        nc, in_maps, core_ids=list(range(NCORES)), trace=trace
    )
    _cache["last_results"] = res

    out = np.empty((B, T, C), dtype=np.float32)
    for b in range(B):
        acc = res.results[b * 4]["y"].astype(np.float32)
        for hg in range(1, 4):
            acc += res.results[b * 4 + hg]["y"].astype(np.float32)
        out[b] = acc.T + bo
    return out

